# revision 4
# baseline (speedup 1.0000x reference)
"""Trainium2 Bass kernel for nn_GAT_27539330301988 (2-layer GAT, N=100k, E=6.4M).

Single-dispatch SPMD strategy (8 NeuronCores):
  - Host does index-only preprocessing: add self loops, sort edges by
    (destination, source), deal nodes round-robin to 8 cores by in-degree
    rank, build per-node padded edge lists (degree-binned superblocks of
    500 nodes, padding repeats the last real entry and is masked out on
    device via a per-node degree compare).
  - Wire-format compression: node features ship as bf16; edge source ids
    ship as a uint16 low-half plane, the 17th bit is reconstructed on
    device from a per-node threshold (sources are sorted, so the high bit
    is monotone along each adjacency list).
  - ONE device dispatch does everything:
      * G1 table build: g1 = [x@W1 | x@W1@As | x@W1@Ad] for own nodes,
        AllGather -> full 100k x 16 table per core.
      * layer-1 edge pass: per-edge indirect gather of 64B table rows,
        masked softmax aggregation (max-subtraction skipped: exact by
        shift invariance, |logit| small), + b1, transposed into a
        resident [40, 3125] activation tile.
      * BN stats: local sum/sumsq folded 40->10 by matmul, AllReduce,
        scale/shift built on device, BN + ELU in place.
      * G2 table build: h1 @ W2eff via a DRAM restage (SBUF partition
        bases must be 0/32/64/96), AllGather -> full table.
      * layer-2 edge pass -> + b2 -> per-core output rows.
  - Host re-assembles the permuted output (bitwise moves only).
"""
import numpy as np
from contextlib import ExitStack

import ml_dtypes
import concourse.bass as bass
import concourse.bacc as bacc
import concourse.tile as tile
from concourse import mybir
from concourse.bass_utils import run_bass_kernel_spmd
from concourse.masks import make_identity

F32 = mybir.dt.float32
I32 = mybir.dt.int32
U16 = mybir.dt.uint16
U8 = mybir.dt.uint8
BF16 = mybir.dt.bfloat16
AX = mybir.AxisListType
OP = mybir.AluOpType
AF = mybir.ActivationFunctionType

N = 100000
E = 6400000
NCORES = 8
IN_CH = 128
P = 125              # nodes per group (partition dim)
GSB = 4              # groups per superblock
NSB = 25             # superblocks per core
NGRP = NSB * GSB     # 100 groups per core
MPC = N // NCORES    # 12500 nodes per core
ROWF = 16            # floats per table row (64B, one HBM burst)
EPS_BN = 1e-5
RG = [list(range(NCORES))]

_nc_cache = {}


def _indirect_q(nc, out, tbl, offset_ap, queue):
    eng = nc.gpsimd
    out_l = eng.lower_ap_dma(out, for_indirect_dma=True)
    in_l = eng.lower_ap_dma(tbl, for_indirect_dma=True)
    off_l = eng.lower_ap_dma(offset_ap)[0]
    in_l[0].dynamic_ap_info = mybir.DynamicAccessPatternInfo(
        c=0,
        actual_ap=out.ap,
        indirect_dim_max_index=tbl.shape[0],
        offset_expr=[
            mybir.DynamicAccessPatternOffsetExpr(
                coef=ROWF,
                aff_expr=mybir.DynamicAccessPatternOffsetExprAffExpr(
                    kind="IndirectArgId", arg_id=1,
                ),
            )
        ],
    )
    in_l.append(off_l)
    return eng.add_instruction(
        mybir.InstDMACopy(
            name=eng.bass.get_next_instruction_name(),
            queue=queue, mode="Copy",
            ins=in_l, outs=out_l, oob_is_err=True,
        )
    )


# ---------------------------------------------------------------- host prep
def _prep(edge_index):
    ei = np.asarray(edge_index).astype(np.int64)
    loop = np.arange(N, dtype=np.int64)
    src = np.concatenate([ei[0], loop])
    dst = np.concatenate([ei[1], loop])
    deg = np.bincount(dst, minlength=N)
    order = np.argsort(-deg, kind="stable")
    pi = np.concatenate([order[k::NCORES] for k in range(NCORES)])
    pos = np.empty(N, np.int64)
    pos[pi] = np.arange(N)
    newdeg = deg[pi]
    D = newdeg.reshape(NCORES, NSB, GSB * P).max(axis=(0, 2)).astype(int)

    psrc = pos[src]
    pdst = pos[dst]
    eorder = np.lexsort((psrc, pdst))     # sort by dst, then src ascending
    ssrc = psrc[eorder].astype(np.int32)
    starts = np.concatenate([[0], np.cumsum(newdeg)])

    lo_cores, ct_cores, dg_cores, own_cores = [], [], [], []
    for k in range(NCORES):
        lo_parts, ct_parts, dg_parts = [], [], []
        for s in range(NSB):
            Ds = int(D[s])
            npos = k * MPC + s * GSB * P + np.arange(GSB * P)
            d = newdeg[npos]
            jj = np.arange(Ds)[None, :]
            m = jj < d[:, None]
            F = np.empty((GSB * P, Ds), np.int32)
            sidx = (starts[npos][:, None] + np.minimum(jj, (d - 1)[:, None]))
            F[:] = ssrc[sidx]             # pads repeat the last real entry
            F[m] = ssrc[(starts[npos][:, None] + jj)[m]]
            ct = (F < 65536).sum(axis=1)  # page threshold per node
            lo_parts.append((F & 0xFFFF).astype(np.uint16)
                            .reshape(GSB, P, Ds).transpose(1, 0, 2)
                            .reshape(P, GSB * Ds))
            ct_parts.append(ct.astype(np.float32)
                            .reshape(GSB, P).T)
            dg_parts.append(d.astype(np.float32).reshape(GSB, P).T)
        lo_cores.append(np.ascontiguousarray(np.concatenate(lo_parts, axis=1)))
        ct_cores.append(np.ascontiguousarray(np.concatenate(ct_parts, axis=1)))
        dg_cores.append(np.ascontiguousarray(np.concatenate(dg_parts, axis=1)))
        g = np.arange(NGRP)[None, :]
        p = np.arange(P)[:, None]
        own_cores.append(
            np.ascontiguousarray((k * MPC + g * P + p).astype(np.int32)))
    return pi, D, lo_cores, ct_cores, dg_cores, own_cores


# ------------------------------------------------------- merged device kernel
def build_merged(D, RW):
    icols = GSB * int(np.sum(D))
    nc = bacc.Bacc(num_devices=NCORES, num_swdge_queues=2)
    xlo = nc.dram_tensor("xlo", [IN_CH, MPC], U8, kind="ExternalInput")
    xhb = nc.dram_tensor("xhb", [IN_CH, MPC // 2], U8, kind="ExternalInput")
    xsh = nc.dram_tensor("xsh", [P, ROWF], F32, kind="ExternalInput")
    w1 = nc.dram_tensor("w1", [IN_CH, 10], F32, kind="ExternalInput")
    w1t = nc.dram_tensor("w1t", [10, IN_CH], F32, kind="ExternalInput")
    asad1 = nc.dram_tensor("asad1", [10, 4], F32, kind="ExternalInput")
    lo16 = nc.dram_tensor("lo16", [P, icols], U16, kind="ExternalInput")
    ctf = nc.dram_tensor("ctf", [P, NGRP], F32, kind="ExternalInput")
    degf = nc.dram_tensor("degf", [P, NGRP], F32, kind="ExternalInput")
    rampf = nc.dram_tensor("rampf", [P, RW], F32, kind="ExternalInput")
    own = nc.dram_tensor("own", [P, NGRP], I32, kind="ExternalInput")
    b1r = nc.dram_tensor("b1r", [P, 10], F32, kind="ExternalInput")
    rep4010 = nc.dram_tensor("rep4010", [40, 10], F32, kind="ExternalInput")
    rep1040 = nc.dram_tensor("rep1040", [10, 40], F32, kind="ExternalInput")
    w2 = nc.dram_tensor("w2", [10, 10], F32, kind="ExternalInput")
    w2t = nc.dram_tensor("w2t", [10, 10], F32, kind="ExternalInput")
    asad2 = nc.dram_tensor("asad2", [10, 2], F32, kind="ExternalInput")
    gmbt = nc.dram_tensor("gmbt", [10, 2], F32, kind="ExternalInput")
    b2r = nc.dram_tensor("b2r", [P, 10], F32, kind="ExternalInput")
    out2 = nc.dram_tensor("out2", [MPC, 10], BF16, kind="ExternalOutput")

    with tile.TileContext(nc) as tc, ExitStack() as ctx:
        dram = ctx.enter_context(tc.tile_pool(name="dram", bufs=1, space="DRAM"))
        res = ctx.enter_context(tc.tile_pool(name="res", bufs=1))
        sb = ctx.enter_context(tc.tile_pool(name="sb", bufs=2))
        ps = ctx.enter_context(tc.tile_pool(name="ps", bufs=2, space="PSUM"))

        g1loc = dram.tile([MPC, ROWF], F32)
        g1full = dram.tile([N, ROWF], F32)
        g2loc = dram.tile([MPC, ROWF], F32)
        g2full = dram.tile([N, ROWF], F32)
        x1t = dram.tile([40, NSB * P], F32)
        bnin = dram.tile([10, 2], F32)
        bnout = dram.tile([10, 2], F32)

        # ---- resident small params / index planes ----
        lot = res.tile([P, icols], U16)
        nc.sync.dma_start(out=lot[:], in_=lo16[:])
        ctt = res.tile([P, NGRP], F32)
        nc.sync.dma_start(out=ctt[:], in_=ctf[:])
        degt = res.tile([P, NGRP], F32)
        nc.sync.dma_start(out=degt[:], in_=degf[:])
        rampt = res.tile([P, RW], F32)
        nc.sync.dma_start(out=rampt[:], in_=rampf[:])
        ownall = res.tile([P, NGRP], I32)
        nc.sync.dma_start(out=ownall[:], in_=own[:])
        b1t = res.tile([P, 10], F32)
        nc.sync.dma_start(out=b1t[:], in_=b1r[:])
        b2t = res.tile([P, 10], F32)
        nc.sync.dma_start(out=b2t[:], in_=b2r[:])
        x1res = res.tile([40, NSB * P], F32)

        # ---- phase A1: own-node G1 rows ----
        a1_pool = tc.tile_pool(name="a1", bufs=1)
        a1 = a1_pool.__enter__()
        w1eff = res.tile([IN_CH, ROWF], F32)
        nc.gpsimd.memset(w1eff[:], 0.0)
        nc.sync.dma_start(out=w1eff[:, 0:10], in_=w1[:])
        w1t_s = a1.tile([10, IN_CH], F32)
        nc.sync.dma_start(out=w1t_s[:], in_=w1t[:])
        asad_s = a1.tile([10, 4], F32)
        nc.sync.dma_start(out=asad_s[:], in_=asad1[:])
        pw = ps.tile([IN_CH, 4], F32, tag="small", bufs=1)
        nc.tensor.matmul(pw[:], lhsT=w1t_s[:], rhs=asad_s[:], start=True, stop=True)
        nc.vector.tensor_copy(out=w1eff[:, 10:14], in_=pw[:])

        xlot = a1.tile([IN_CH, MPC], U8)
        nc.sync.dma_start(out=xlot[:], in_=xlo[:])
        xhbt = a1.tile([IN_CH, MPC // 2], U8)
        nc.sync.dma_start(out=xhbt[:], in_=xhb[:])
        xsht = a1.tile([P, ROWF], F32)
        nc.sync.dma_start(out=xsht[:], in_=xsh[:])
        CH = 500
        CH2 = CH // 2
        for c in range(MPC // CH):
            # int12 unpack: v = lo + 256*nibble (nibbles packed in pairs)
            xf = sb.tile([IN_CH, CH], F32, tag="xf")
            nc.vector.tensor_copy(out=xf[:], in_=xlot[:, c * CH:(c + 1) * CH])
            hbi = sb.tile([IN_CH, CH2], I32, tag="hbi")
            nc.vector.tensor_copy(out=hbi[:], in_=xhbt[:, c * CH2:(c + 1) * CH2])
            hi0 = sb.tile([IN_CH, CH2], I32, tag="hi0")
            nc.vector.tensor_scalar(out=hi0[:], in0=hbi[:], scalar1=15,
                                    scalar2=None, op0=OP.bitwise_and)
            nc.vector.tensor_scalar(out=hbi[:], in0=hbi[:], scalar1=4,
                                    scalar2=None, op0=OP.logical_shift_right)
            hi0f = sb.tile([IN_CH, CH2], F32, tag="hi0f")
            nc.vector.tensor_copy(out=hi0f[:], in_=hi0[:])
            hi1f = sb.tile([IN_CH, CH2], F32, tag="hi1f")
            nc.vector.tensor_copy(out=hi1f[:], in_=hbi[:])
            nc.vector.tensor_scalar(out=hi0f[:], in0=hi0f[:], scalar1=256.0,
                                    scalar2=None, op0=OP.mult)
            nc.vector.tensor_scalar(out=hi1f[:], in0=hi1f[:], scalar1=256.0,
                                    scalar2=None, op0=OP.mult)
            xf3 = xf[:].rearrange("c (j t) -> c j t", t=2)
            nc.vector.tensor_tensor(out=xf3[:, :, 0:1], in0=xf3[:, :, 0:1],
                                    in1=hi0f[:].unsqueeze(2), op=OP.add)
            nc.vector.tensor_tensor(out=xf3[:, :, 1:2], in0=xf3[:, :, 1:2],
                                    in1=hi1f[:].unsqueeze(2), op=OP.add)
            for t in range(CH // P):
                pt = ps.tile([P, ROWF], F32, tag="pt")
                nc.tensor.matmul(pt[:], lhsT=xf[:, t * P:(t + 1) * P],
                                 rhs=w1eff[:], start=True, stop=True)
                row = sb.tile([P, ROWF], F32, tag="row")
                nc.vector.tensor_tensor(out=row[:], in0=pt[:], in1=xsht[:],
                                        op=OP.subtract)
                a = c * CH + t * P
                nc.sync.dma_start(out=g1loc[a:a + P, :], in_=row[:])
        a1_pool.__exit__(None, None, None)

        tc.strict_bb_all_engine_barrier()
        nc.gpsimd.collective_compute(
            "AllGather", OP.bypass, replica_groups=RG,
            ins=[g1loc[:].opt()], outs=[g1full[:].opt()])

        idt = res.tile([P, P], F32)
        make_identity(nc, idt[:])

        # ---- layer-1 edge pass ----
        def build_idx_mask(s, Ds, coff, heads):
            """Reconstruct int32 gather indices + real-edge mask for sb s."""
            W = GSB * Ds
            lof = sb.tile([P, W], F32, tag="lof")
            nc.vector.tensor_copy(out=lof[:], in_=lot[:, coff:coff + W])
            page = sb.tile([P, W], F32, tag="page")
            page3 = page[:].rearrange("p (g d) -> p g d", g=GSB)
            nc.vector.tensor_tensor(
                out=page3[:, :, :],
                in0=rampt[:].unsqueeze(1)[:, :, 0:Ds].broadcast_to([P, GSB, Ds]),
                in1=ctt[:, s * GSB:(s + 1) * GSB].unsqueeze(2)
                    .broadcast_to([P, GSB, Ds]),
                op=OP.is_ge)
            nc.vector.tensor_scalar(out=page[:], in0=page[:], scalar1=65536.0,
                                    scalar2=None, op0=OP.mult)
            nc.vector.tensor_tensor(out=lof[:], in0=lof[:], in1=page[:], op=OP.add)
            idxt = sb.tile([P, W], I32, tag="idxt")
            nc.vector.tensor_copy(out=idxt[:], in_=lof[:])
            mask = sb.tile([P, W * heads], F32, tag="mask")
            mask3 = mask[:].rearrange("p (g d h) -> p g d h", g=GSB, h=heads)
            nc.vector.tensor_tensor(
                out=mask3[:, :, :, :],
                in0=rampt[:].unsqueeze(1).unsqueeze(3)[:, :, 0:Ds, :]
                    .broadcast_to([P, GSB, Ds, heads]),
                in1=degt[:, s * GSB:(s + 1) * GSB].unsqueeze(2).unsqueeze(3)
                    .broadcast_to([P, GSB, Ds, heads]),
                op=OP.is_lt)
            return idxt, mask

        def gather(tbl, idxt, s, Ds):
            W = GSB * Ds
            qn = "qPoolDynamic" if s % 2 == 0 else "qPoolDynamic1"
            g = sb.tile([P, W * ROWF], F32, tag="g")
            for j in range(W):
                _indirect_q(nc, g[:, j * ROWF:(j + 1) * ROWF], tbl[:],
                            idxt[:, j:j + 1], qn)
            o = sb.tile([P, GSB * ROWF], F32, tag="o")
            for j in range(GSB):
                nc.gpsimd.indirect_dma_start(
                    out=o[:, j * ROWF:(j + 1) * ROWF], out_offset=None,
                    in_=tbl[:],
                    in_offset=bass.IndirectOffsetOnAxis(
                        ap=ownall[:, GSB * s + j:GSB * s + j + 1], axis=0))
            return g, o

        coff = 0
        for s in range(NSB):
            Ds = int(D[s])
            idxt, mask = build_idx_mask(s, Ds, coff, 2)
            g, o = gather(g1full, idxt, s, Ds)
            coff += GSB * Ds

            g4 = g[:].rearrange("p (g d c) -> p g d c", g=GSB, c=ROWF)
            o3 = o[:].rearrange("p (g c) -> p g c", c=ROWF)
            ex = sb.tile([P, GSB * Ds * 2], F32, tag="ex")
            ex4 = ex[:].rearrange("p (g d h) -> p g d h", g=GSB, h=2)
            nc.vector.tensor_tensor(
                out=ex4[:, :, :, :], in0=g4[:, :, :, 10:12],
                in1=o3[:, :, None, 12:14].broadcast_to([P, GSB, Ds, 2]),
                op=OP.add)
            ext = sb.tile([P, GSB * Ds * 2], F32, tag="ext")
            nc.vector.tensor_scalar(out=ext[:], in0=ex[:], scalar1=0.2,
                                    scalar2=None, op0=OP.mult)
            nc.vector.tensor_tensor(out=ex[:], in0=ex[:], in1=ext[:], op=OP.max)
            nc.scalar.activation(out=ex[:], in_=ex[:], func=AF.Exp)
            nc.vector.tensor_tensor(out=ex[:], in0=ex[:], in1=mask[:], op=OP.mult)

            msg = sb.tile([P, GSB * Ds * 10], F32, tag="msg")
            msg4 = msg[:].rearrange("p (g d c) -> p g d c", g=GSB, c=10)
            for h in range(2):
                nc.vector.tensor_tensor(
                    out=msg4[:, :, :, 5 * h:5 * h + 5],
                    in0=g4[:, :, :, 5 * h:5 * h + 5],
                    in1=ex4[:, :, :, h:h + 1].broadcast_to([P, GSB, Ds, 5]),
                    op=OP.mult)

            accm = sb.tile([P, GSB * 10], F32, tag="accm")
            nc.vector.tensor_reduce(
                out=accm[:].rearrange("p (g c) -> p g c", g=GSB),
                in_=msg[:].rearrange("p (g d c) -> p g c d", g=GSB, c=10),
                axis=AX.X, op=OP.add)
            acce = sb.tile([P, GSB * 2], F32, tag="acce")
            nc.vector.tensor_reduce(
                out=acce[:].rearrange("p (g h) -> p g h", g=GSB),
                in_=ex[:].rearrange("p (g d h) -> p g h d", g=GSB, h=2),
                axis=AX.X, op=OP.add)
            nc.vector.tensor_scalar(out=acce[:], in0=acce[:], scalar1=1e-16,
                                    scalar2=None, op0=OP.add)
            nc.vector.reciprocal(out=acce[:], in_=acce[:])

            o1 = sb.tile([P, GSB * 10], F32, tag="o1")
            o1v = o1[:].rearrange("p (g h c) -> p g h c", g=GSB, h=2)
            nc.vector.tensor_tensor(
                out=o1v[:, :, :, :],
                in0=accm[:].rearrange("p (g h c) -> p g h c", g=GSB, h=2),
                in1=acce[:].rearrange("p (g h) -> p g h", g=GSB)
                    [:, :, :, None].broadcast_to([P, GSB, 2, 5]),
                op=OP.mult)
            nc.vector.tensor_tensor(
                out=o1[:].rearrange("p (g c) -> p g c", g=GSB),
                in0=o1[:].rearrange("p (g c) -> p g c", g=GSB),
                in1=b1t[:].unsqueeze(1).broadcast_to([P, GSB, 10]),
                op=OP.add)

            pst = ps.tile([GSB * 10, P], F32, tag="pst")
            nc.tensor.transpose(out=pst[:], in_=o1[:], identity=idt[:])
            nc.vector.tensor_copy(out=x1res[:, s * P:(s + 1) * P], in_=pst[:])

        # ---- BN stats + AllReduce ----
        st_pool = tc.tile_pool(name="st", bufs=1)
        stp = st_pool.__enter__()
        sq = stp.tile([40, NSB * P], F32)
        nc.vector.tensor_tensor(out=sq[:], in0=x1res[:], in1=x1res[:], op=OP.mult)
        st2 = stp.tile([40, 2], F32, tag="st2")
        nc.vector.tensor_reduce(out=st2[:, 0:1], in_=x1res[:], axis=AX.X, op=OP.add)
        nc.vector.tensor_reduce(out=st2[:, 1:2], in_=sq[:], axis=AX.X, op=OP.add)
        rep40t = stp.tile([40, 10], F32, tag="rep40t")
        nc.sync.dma_start(out=rep40t[:], in_=rep4010[:])
        pf = ps.tile([10, 2], F32, tag="small", bufs=1)
        nc.tensor.matmul(pf[:], lhsT=rep40t[:], rhs=st2[:], start=True, stop=True)
        stl = stp.tile([10, 2], F32, tag="stl")
        nc.vector.tensor_copy(out=stl[:], in_=pf[:])
        nc.sync.dma_start(out=bnin[:], in_=stl[:])
        tc.strict_bb_all_engine_barrier()
        nc.gpsimd.collective_compute(
            "AllReduce", OP.add, replica_groups=RG,
            ins=[bnin[:].opt()], outs=[bnout[:].opt()])
        stg = stp.tile([10, 2], F32, tag="stg")
        nc.gpsimd.dma_start(out=stg[:], in_=bnout[:])

        mm = stp.tile([10, 2], F32, tag="mm")
        nc.vector.tensor_scalar(out=mm[:], in0=stg[:], scalar1=1.0 / N,
                                scalar2=None, op0=OP.mult)
        var = stp.tile([10, 1], F32, tag="var")
        nc.vector.tensor_tensor(out=var[:], in0=mm[:, 0:1], in1=mm[:, 0:1],
                                op=OP.mult)
        nc.vector.tensor_tensor(out=var[:], in0=mm[:, 1:2], in1=var[:],
                                op=OP.subtract)
        nc.vector.tensor_scalar(out=var[:], in0=var[:], scalar1=EPS_BN,
                                scalar2=None, op0=OP.add)
        nc.vector.reciprocal(out=var[:], in_=var[:])
        rstd = stp.tile([10, 1], F32, tag="rstd")
        nc.scalar.activation(out=rstd[:], in_=var[:], func=AF.Sqrt)
        gb = stp.tile([10, 2], F32, tag="gb")
        nc.sync.dma_start(out=gb[:], in_=gmbt[:])
        sc2 = stp.tile([10, 2], F32, tag="sc2")
        nc.vector.tensor_tensor(out=sc2[:, 0:1], in0=rstd[:], in1=gb[:, 0:1],
                                op=OP.mult)
        nc.vector.tensor_tensor(out=sc2[:, 1:2], in0=mm[:, 0:1], in1=sc2[:, 0:1],
                                op=OP.mult)
        nc.vector.tensor_tensor(out=sc2[:, 1:2], in0=gb[:, 1:2], in1=sc2[:, 1:2],
                                op=OP.subtract)
        rep10t = stp.tile([10, 40], F32, tag="rep10t")
        nc.sync.dma_start(out=rep10t[:], in_=rep1040[:])
        pr = ps.tile([40, 2], F32, tag="small", bufs=1)
        nc.tensor.matmul(pr[:], lhsT=rep10t[:], rhs=sc2[:], start=True, stop=True)
        ssr = stp.tile([40, 2], F32, tag="ssr")
        nc.vector.tensor_copy(out=ssr[:], in_=pr[:])

        # ---- BN + ELU in place on x1res ----
        nc.vector.tensor_scalar(out=x1res[:], in0=x1res[:], scalar1=ssr[:, 0:1],
                                scalar2=ssr[:, 1:2], op0=OP.mult, op1=OP.add)
        nc.vector.tensor_scalar(out=sq[:], in0=x1res[:], scalar1=0.0,
                                scalar2=None, op0=OP.min)
        nc.scalar.activation(out=sq[:], in_=sq[:], func=AF.Exp)
        nc.vector.tensor_scalar(out=sq[:], in0=sq[:], scalar1=-1.0,
                                scalar2=None, op0=OP.add)
        nc.vector.tensor_tensor(out=x1res[:], in0=x1res[:], in1=sq[:], op=OP.max)
        nc.sync.dma_start(out=x1t[:], in_=x1res[:])
        st_pool.__exit__(None, None, None)
        tc.strict_bb_all_engine_barrier()

        # ---- W2eff + G2 table build ----
        g2_pool = tc.tile_pool(name="g2p", bufs=2)
        g2p = g2_pool.__enter__()
        w2eff = res.tile([10, ROWF], F32)
        nc.gpsimd.memset(w2eff[:], 0.0)
        nc.sync.dma_start(out=w2eff[:, 0:10], in_=w2[:])
        w2t_s = g2p.tile([10, 10], F32, tag="w2ts")
        nc.sync.dma_start(out=w2t_s[:], in_=w2t[:])
        asad2_s = g2p.tile([10, 2], F32, tag="asad2")
        nc.sync.dma_start(out=asad2_s[:], in_=asad2[:])
        pw2 = ps.tile([10, 2], F32, tag="small", bufs=1)
        nc.tensor.matmul(pw2[:], lhsT=w2t_s[:], rhs=asad2_s[:], start=True, stop=True)
        nc.vector.tensor_copy(out=w2eff[:, 10:12], in_=pw2[:])

        for g in range(GSB):
            stage = g2p.tile([10, NSB * P], F32, tag="stage")
            nc.sync.dma_start(out=stage[:], in_=x1t[g * 10:(g + 1) * 10, :])
            for s in range(NSB):
                pt = ps.tile([P, ROWF], F32, tag="pt")
                nc.tensor.matmul(pt[:], lhsT=stage[:, s * P:(s + 1) * P],
                                 rhs=w2eff[:], start=True, stop=True)
                rt = sb.tile([P, ROWF], F32, tag="rt")
                nc.vector.tensor_copy(out=rt[:], in_=pt[:])
                grp = s * GSB + g
                nc.sync.dma_start(out=g2loc[grp * P:(grp + 1) * P, :], in_=rt[:])
        g2_pool.__exit__(None, None, None)

        tc.strict_bb_all_engine_barrier()
        nc.gpsimd.collective_compute(
            "AllGather", OP.bypass, replica_groups=RG,
            ins=[g2loc[:].opt()], outs=[g2full[:].opt()])

        # ---- layer-2 edge pass ----
        coff = 0
        for s in range(NSB):
            Ds = int(D[s])
            idxt, mask = build_idx_mask(s, Ds, coff, 1)
            g, o = gather(g2full, idxt, s, Ds)
            coff += GSB * Ds

            g4 = g[:].rearrange("p (g d c) -> p g d c", g=GSB, c=ROWF)
            o3 = o[:].rearrange("p (g c) -> p g c", c=ROWF)
            ex = sb.tile([P, GSB * Ds], F32, tag="ex")
            ex3 = ex[:].rearrange("p (g d) -> p g d", g=GSB)
            nc.vector.tensor_tensor(
                out=ex3[:, :, :], in0=g4[:, :, :, 10],
                in1=o3[:, :, 11:12].broadcast_to([P, GSB, Ds]),
                op=OP.add)
            ext = sb.tile([P, GSB * Ds], F32, tag="ext")
            nc.vector.tensor_scalar(out=ext[:], in0=ex[:], scalar1=0.2,
                                    scalar2=None, op0=OP.mult)
            nc.vector.tensor_tensor(out=ex[:], in0=ex[:], in1=ext[:], op=OP.max)
            nc.scalar.activation(out=ex[:], in_=ex[:], func=AF.Exp)
            nc.vector.tensor_tensor(out=ex[:], in0=ex[:], in1=mask[:], op=OP.mult)

            msg = sb.tile([P, GSB * Ds * 10], F32, tag="msg")
            msg4 = msg[:].rearrange("p (g d c) -> p g d c", g=GSB, c=10)
            nc.vector.tensor_tensor(
                out=msg4[:, :, :, :],
                in0=g4[:, :, :, 0:10],
                in1=ex3[:, :, :, None].broadcast_to([P, GSB, Ds, 10]),
                op=OP.mult)

            accm = sb.tile([P, GSB * 10], F32, tag="accm")
            nc.vector.tensor_reduce(
                out=accm[:].rearrange("p (g c) -> p g c", g=GSB),
                in_=msg[:].rearrange("p (g d c) -> p g c d", g=GSB, c=10),
                axis=AX.X, op=OP.add)
            acce = sb.tile([P, GSB], F32, tag="acce")
            nc.vector.tensor_reduce(
                out=acce[:],
                in_=ex[:].rearrange("p (g d) -> p g d", g=GSB),
                axis=AX.X, op=OP.add)
            nc.vector.tensor_scalar(out=acce[:], in0=acce[:], scalar1=1e-16,
                                    scalar2=None, op0=OP.add)
            nc.vector.reciprocal(out=acce[:], in_=acce[:])

            o2 = sb.tile([P, GSB * 10], F32, tag="o2")
            o2v = o2[:].rearrange("p (g c) -> p g c", g=GSB)
            nc.vector.tensor_tensor(
                out=o2v[:, :, :],
                in0=accm[:].rearrange("p (g c) -> p g c", g=GSB),
                in1=acce[:].unsqueeze(2).broadcast_to([P, GSB, 10]),
                op=OP.mult)
            nc.vector.tensor_tensor(
                out=o2v[:, :, :], in0=o2v[:, :, :],
                in1=b2t[:].unsqueeze(1).broadcast_to([P, GSB, 10]),
                op=OP.add)
            o2b = sb.tile([P, GSB * 10], BF16, tag="o2b")
            nc.vector.tensor_copy(out=o2b[:], in_=o2[:])
            nc.sync.dma_start(
                out=out2[s * GSB * P:(s + 1) * GSB * P, :].rearrange(
                    "(g p) c -> p g c", p=P),
                in_=o2b[:].rearrange("p (g c) -> p g c", g=GSB))
    nc.compile()
    return nc


# ---------------------------------------------------------------- driver
def kernel(x, W1, a_src1, a_dst1, b1, gamma1, beta1, W2, a_src2, a_dst2, b2,
           edge_index):
    x = np.ascontiguousarray(np.asarray(x, dtype=np.float32))
    W1 = np.asarray(W1, np.float32)
    W2 = np.asarray(W2, np.float32)
    a_src1 = np.asarray(a_src1, np.float32)
    a_dst1 = np.asarray(a_dst1, np.float32)
    a_src2 = np.asarray(a_src2, np.float32)
    a_dst2 = np.asarray(a_dst2, np.float32)
    b1 = np.asarray(b1, np.float32)
    b2 = np.asarray(b2, np.float32)
    gamma1 = np.asarray(gamma1, np.float32)
    beta1 = np.asarray(beta1, np.float32)

    pi, D, lo_cores, ct_cores, dg_cores, own_cores = _prep(edge_index)
    RW = max(128, int(D.max()))
    cores = list(range(NCORES))

    s12 = 2047.0 / max(np.abs(x).max(), 1e-30)
    v12 = (np.rint(x.T * s12).clip(-2047, 2047) + 2048.0).astype(np.uint16)
    w1s = (W1 / s12).astype(np.float32)
    asad1 = np.zeros((10, 4), np.float32)
    for h in range(2):
        asad1[5 * h:5 * h + 5, h] = a_src1[h]
        asad1[5 * h:5 * h + 5, 2 + h] = a_dst1[h]
    w1tc = np.ascontiguousarray(w1s.T)
    w1eff_h = np.zeros((IN_CH, ROWF), np.float32)
    w1eff_h[:, 0:10] = w1s
    w1eff_h[:, 10:14] = w1s @ asad1
    xsh_row = 2048.0 * w1eff_h.sum(axis=0)
    xsh = np.ascontiguousarray(np.tile(xsh_row, (P, 1)))
    b1r = np.ascontiguousarray(np.tile(b1, (P, 1)))
    b2r = np.ascontiguousarray(np.tile(b2, (P, 1)))
    rampf = np.ascontiguousarray(
        np.broadcast_to(np.arange(RW, dtype=np.float32), (P, RW)))
    rep4010 = np.zeros((40, 10), np.float32)
    rep4010[np.arange(40), np.arange(40) % 10] = 1.0
    rep1040 = np.ascontiguousarray(rep4010.T)
    asad2 = np.zeros((10, 2), np.float32)
    asad2[:, 0] = a_src2[0]
    asad2[:, 1] = a_dst2[0]
    w2tc = np.ascontiguousarray(W2.T)
    gmbt = np.ascontiguousarray(np.stack([gamma1, beta1], axis=1))

    in_maps = []
    for k in cores:
        vk = v12[:, pi[k * MPC:(k + 1) * MPC]]
        hik = (vk >> 8).astype(np.uint8)
        in_maps.append({
            "xlo": np.ascontiguousarray((vk & 255).astype(np.uint8)),
            "xhb": np.ascontiguousarray(
                (hik[:, 0::2] | (hik[:, 1::2] << 4)).astype(np.uint8)),
            "xsh": xsh,
            "w1": w1s, "w1t": w1tc, "asad1": asad1,
            "lo16": lo_cores[k], "ctf": ct_cores[k], "degf": dg_cores[k],
            "rampf": rampf, "own": own_cores[k], "b1r": b1r,
            "rep4010": rep4010, "rep1040": rep1040,
            "w2": W2, "w2t": w2tc, "asad2": asad2, "gmbt": gmbt, "b2r": b2r,
        })

    key = tuple(D.tolist()) + (RW,)
    nc = _nc_cache.get(key)
    if nc is None:
        nc = build_merged(D, RW)
        _nc_cache[key] = nc
    r = run_bass_kernel_spmd(nc, in_maps, cores)

    out = np.empty((N, 10), np.float32)
    shards = np.concatenate(
        [np.asarray(r.results[k]["out2"]).astype(np.float32) for k in cores],
        axis=0)
    out[pi] = shards
    return out
